# revision 28
# baseline (speedup 1.0000x reference)
"""Trainium2 Bass kernel for nn_ETypePromptModel: logits = einsum('bpd,cpd->bc').

Equivalent to X @ W.T with X=[B, L*D]=[16384, 256], W=[C, L*D]=[4096, 256].
Data-parallel over B across 8 NeuronCores; label2embed replicated.

Per-core plan (B_LOC=2048), ~112-117us/core measured (DMA-byte bound:
39.5 MB of DRAM traffic per core at the ~420 GB/s sustained fabric rate):
  - All input loads triggered up front: W chunks 0/1 first on the two
    HWDGE rings (sync/scalar), then X in 4 chunks; each ring's FIFO
    defers the W2/W3 tail behind the data the pipeline start needs.
  - PE-transpose X and W into K-major float32r SBUF layout (fp32 has no
    DMA-transpose path); 4 transposes batched per PSUM bank (4 banks) ->
    one [128, 2, 2, 128] strided copy each on the Vector engine. Only the
    start-critical batches (W0, W1, X m-tiles 0-3) run before the matmul
    stream; the rest interleave into it after their stage DMAs land.
  - 256 float32r matmuls ([128k x 128b] stationary, [128k x 512c] moving,
    1 cycle/row vs 4 for plain fp32), K=256 accumulated over 2 PSUM
    passes; groups of 2 PSUM banks (4 banks), chunk-pair-outer stream.
  - PSUM -> SBUF output copies alternate Scalar/Vector; 32 x 1MB HWDGE
    DMA writes (8KB-contiguous rows) of the [2048, 4096] fp32 output
    slice; first write fires ~25us in, stream sustains ~420-427 GB/s.
"""

import sys

import numpy as np

sys.path.insert(0, "/opt/trn_rl_repo")

B, C, L, D = 16384, 4096, 2, 128
N_CORES = 8
B_LOC = B // N_CORES  # 2048
P = 128
N_TILE = 512  # moving free dim per matmul
M_TILES = B_LOC // P  # 16
C_TILES = C // P  # 32
W_CHUNKS = 4
C_CHUNK = C // W_CHUNKS  # 1024 classes per chunk
N_GROUP = 2  # PSUM banks per matmul accumulation group

_CACHE = {}
PROFILE = False
TRACE_ALL_CORES = False
LAST_RESULT = None


def _build():
    import concourse.mybir as mybir
    import concourse.tile as tile
    from concourse import bacc
    from concourse.masks import make_identity

    f32 = mybir.dt.float32
    f32r = mybir.dt.float32r

    nc = bacc.Bacc(
        "TRN2",
        target_bir_lowering=False,
        debug=False,
        enable_asserts=False,
        num_devices=N_CORES,
    )

    x_dram = nc.dram_tensor("batchs", [B_LOC, L, D], f32, kind="ExternalInput").ap()
    w_dram = nc.dram_tensor("label2embed", [C, L, D], f32, kind="ExternalInput").ap()
    out_dram = nc.dram_tensor("out", [B_LOC, C], f32, kind="ExternalOutput").ap()

    with tile.TileContext(nc) as tc:
        with (
            tc.tile_pool(name="const", bufs=1) as const_pool,
            tc.tile_pool(name="big", bufs=1) as big_pool,
            tc.tile_pool(name="osb", bufs=8) as out_pool,
            tc.tile_pool(name="pst", bufs=4, space="PSUM") as psum_t,
            tc.tile_pool(name="psm", bufs=4, space="PSUM") as psum_mm,
        ):
            ident = const_pool.tile([P, P], f32, name="ident")
            make_identity(nc, ident)

            _cp = [0]

            def copy(out_ap, in_ap):
                if _cp[0] % 2 == 0:
                    nc.scalar.copy(out_ap, in_ap)
                else:
                    nc.vector.tensor_copy(out=out_ap, in_=in_ap)
                _cp[0] += 1

            # ---- bulk input loads ----
            # X first on both HWDGE rings (4 chunks of 4 m-tiles), then W
            # chunks 0/1; W chunks 2/3 are triggered mid-stream so early DMA
            # bandwidth goes to the data the pipeline start needs.
            XQ = 4  # m-tiles per X chunk
            CO = C_TILES // W_CHUNKS  # 8 c-tiles per chunk
            x_stages = [
                big_pool.tile([P, XQ, L, D], f32, name=f"x_stage{xi}")
                for xi in range(M_TILES // XQ)
            ]
            w_engs = (nc.sync, nc.scalar, nc.sync, nc.scalar)
            w_stages = [
                big_pool.tile([P, CO, L, D], f32, name=f"w_stage{ci}")
                for ci in range(W_CHUNKS)
            ]

            def load_x_chunk(xi, eng):
                eng.dma_start(
                    x_stages[xi],
                    x_dram[xi * XQ * P : (xi + 1) * XQ * P].rearrange(
                        "(mo bi) p d -> bi mo p d", bi=P
                    ),
                )

            def load_w_chunk(ci):
                w_engs[ci].dma_start(
                    w_stages[ci],
                    w_dram[ci * CO * P : (ci + 1) * CO * P].rearrange(
                        "(co bi) p d -> bi co p d", bi=P
                    ),
                )

            # All loads up front; each ring's FIFO defers the low-priority
            # tail (W2/W3) behind the data the pipeline start needs.
            load_w_chunk(0)
            load_w_chunk(1)
            load_x_chunk(0, nc.sync)
            load_x_chunk(1, nc.scalar)
            load_x_chunk(2, nc.sync)
            load_x_chunk(3, nc.scalar)
            load_w_chunk(2)
            load_w_chunk(3)

            # ---- transposes ----
            # 4 [128,128] PE transposes batched into one PSUM bank, then one
            # [128, 2, 2, 128] strided copy out (cast to f32r).
            def transpose_batch(dst, dst_off, src, src_off, tag, alternate=False):
                ps = psum_t.tile([P, 2, L, P], f32, tag="tps", name=tag)
                for m1 in range(2):
                    for p in range(L):
                        nc.tensor.transpose(
                            ps[:, m1, p, :], src[:, src_off + m1, p, :], ident
                        )
                dst_ap = dst[:, :, dst_off : dst_off + 2 * P].rearrange(
                    "d p (m b) -> d p m b", m=2
                )
                src_ap = ps.rearrange("d m p b -> d p m b")
                if alternate == "scalar":
                    nc.scalar.copy(dst_ap, src_ap)
                else:
                    nc.vector.tensor_copy(out=dst_ap, in_=src_ap)

            # W.T per chunk: wt_chunks[ci][d, p, c'] = W[ci*1024 + c', p, d]
            wt_chunks = [
                big_pool.tile([P, L, C_CHUNK], f32r, name=f"wt{ci}")
                for ci in range(W_CHUNKS)
            ]

            def w_transpose_batch(ci, co2, alternate=False):
                transpose_batch(
                    wt_chunks[ci],
                    co2 * 2 * P,
                    w_stages[ci],
                    co2 * 2,
                    "tps_w",
                    alternate=alternate,
                )

            # chunks 0 and 1 transposed up front (they land first); both
            # copy engines are idle pre-stream, so alternate them here
            for co2 in range(CO // 2):
                w_transpose_batch(0, co2)
            for co2 in range(CO // 2):
                w_transpose_batch(1, co2, alternate="scalar")

            # X.T per chunk: xt_chunks[q][d, p, b'] = X[q*512 + b', p, d]
            xt_chunks = [
                big_pool.tile([P, L, XQ * P], f32r, name=f"xt{xi}")
                for xi in range(M_TILES // XQ)
            ]

            def x_transpose_batch(mo2, alternate=False):
                xi = mo2 * 2 // XQ
                transpose_batch(
                    xt_chunks[xi],
                    ((mo2 * 2) % XQ) * P,
                    x_stages[xi],
                    (mo2 * 2) % XQ,
                    "tps_x",
                    alternate=alternate,
                )

            # only chunk 0 of X (m-tiles 0-3) before the stream; the rest
            # interleave into the early matmul stream below
            x_transpose_batch(0)
            x_transpose_batch(1)

            # ---- main matmul stream: chunk-pair-outer (8KB output rows) ----
            for cpair in range(W_CHUNKS // 2):
                for mt in range(M_TILES):
                    if cpair == 0:
                        # X chunks 1-3 transposes early in the stream (each
                        # well after its stage DMA lands, before first use at
                        # mt 4/8/12); W chunks 2,3 in the back half.
                        if 1 <= mt <= 3:
                            x_transpose_batch(mt * 2)
                            x_transpose_batch(mt * 2 + 1)
                        if mt >= 8:
                            w_transpose_batch(2 + (mt - 8) // 4, (mt - 8) % 4)

                    out_sb = out_pool.tile(
                        [P, 2 * C_CHUNK], f32, tag="osb", name="out_sb"
                    )
                    for sub in range(2):
                        ci = cpair * 2 + sub
                        wt = wt_chunks[ci]
                        pms = [
                            psum_mm.tile([P, N_TILE], f32, tag="pmm", name="pmm")
                            for _ in range(N_GROUP)
                        ]
                        for p in range(L):
                            for j in range(N_GROUP):
                                nc.tensor.matmul(
                                    pms[j],
                                    xt_chunks[mt // XQ][
                                        :, p, (mt % XQ) * P : (mt % XQ + 1) * P
                                    ],
                                    wt[:, p, j * N_TILE : (j + 1) * N_TILE],
                                    start=(p == 0),
                                    stop=(p == L - 1),
                                )
                        for j in range(N_GROUP):
                            copy(
                                out_sb[
                                    :,
                                    sub * C_CHUNK
                                    + j * N_TILE : sub * C_CHUNK
                                    + (j + 1) * N_TILE,
                                ],
                                pms[j],
                            )
                    nc.sync.dma_start(
                        out_dram[
                            mt * P : (mt + 1) * P,
                            cpair * 2 * C_CHUNK : (cpair + 1) * 2 * C_CHUNK,
                        ],
                        out_sb,
                    )

    nc.compile()
    return nc


def kernel(batchs, label2embed):
    global LAST_RESULT
    from concourse.bass_utils import run_bass_kernel_spmd

    if "nc" not in _CACHE:
        _CACHE["nc"] = _build()
    nc = _CACHE["nc"]

    batchs = np.ascontiguousarray(batchs, dtype=np.float32)
    label2embed = np.ascontiguousarray(label2embed, dtype=np.float32)
    assert batchs.shape == (B, L, D) and label2embed.shape == (C, L, D)

    in_maps = [
        {
            "batchs": batchs[c * B_LOC : (c + 1) * B_LOC],
            "label2embed": label2embed,
        }
        for c in range(N_CORES)
    ]
    res = run_bass_kernel_spmd(
        nc,
        in_maps,
        core_ids=list(range(N_CORES)),
        trace=PROFILE,
        trace_cores=list(range(N_CORES)) if (PROFILE and TRACE_ALL_CORES) else None,
    )
    LAST_RESULT = res
    return np.concatenate([r["out"] for r in res.results], axis=0)


# revision 29
# speedup vs baseline: 1.0003x; 1.0003x over previous
"""Trainium2 Bass kernel for nn_ETypePromptModel: logits = einsum('bpd,cpd->bc').

Equivalent to X @ W.T with X=[B, L*D]=[16384, 256], W=[C, L*D]=[4096, 256].
Data-parallel over B across 8 NeuronCores; label2embed replicated.

Per-core plan (B_LOC=2048), ~112-117us/core measured (DMA-byte bound:
39.5 MB of DRAM traffic per core at the ~420 GB/s sustained fabric rate):
  - All input loads triggered up front: W chunks 0/1 first on the two
    HWDGE rings (sync/scalar), then X in 4 chunks; each ring's FIFO
    defers the W2/W3 tail behind the data the pipeline start needs.
  - PE-transpose X and W into K-major float32r SBUF layout (fp32 has no
    DMA-transpose path); 4 transposes batched per PSUM bank (4 banks) ->
    one [128, 2, 2, 128] strided copy each on the Vector engine. Only the
    start-critical batches (W0, W1, X m-tiles 0-3) run before the matmul
    stream; the rest interleave into it after their stage DMAs land.
  - 256 float32r matmuls ([128k x 128b] stationary, [128k x 512c] moving,
    1 cycle/row vs 4 for plain fp32), K=256 accumulated over 2 PSUM
    passes; groups of 2 PSUM banks (4 banks), chunk-pair-outer stream.
  - PSUM -> SBUF output copies alternate Scalar/Vector; 32 x 1MB HWDGE
    DMA writes (8KB-contiguous rows) of the [2048, 4096] fp32 output
    slice; first write fires ~25us in, stream sustains ~420-427 GB/s.
"""

import sys

import numpy as np

sys.path.insert(0, "/opt/trn_rl_repo")

B, C, L, D = 16384, 4096, 2, 128
N_CORES = 8
B_LOC = B // N_CORES  # 2048
P = 128
N_TILE = 512  # moving free dim per matmul
M_TILES = B_LOC // P  # 16
C_TILES = C // P  # 32
W_CHUNKS = 4
C_CHUNK = C // W_CHUNKS  # 1024 classes per chunk
N_GROUP = 2  # PSUM banks per matmul accumulation group

_CACHE = {}
PROFILE = False
TRACE_ALL_CORES = False
LAST_RESULT = None


def _build():
    import concourse.mybir as mybir
    import concourse.tile as tile
    from concourse import bacc
    from concourse.masks import make_identity

    f32 = mybir.dt.float32
    f32r = mybir.dt.float32r

    nc = bacc.Bacc(
        "TRN2",
        target_bir_lowering=False,
        debug=False,
        enable_asserts=False,
        num_devices=N_CORES,
    )

    x_dram = nc.dram_tensor("batchs", [B_LOC, L, D], f32, kind="ExternalInput").ap()
    w_dram = nc.dram_tensor("label2embed", [C, L, D], f32, kind="ExternalInput").ap()
    out_dram = nc.dram_tensor("out", [B_LOC, C], f32, kind="ExternalOutput").ap()

    with tile.TileContext(nc) as tc:
        with (
            tc.tile_pool(name="const", bufs=1) as const_pool,
            tc.tile_pool(name="big", bufs=1) as big_pool,
            tc.tile_pool(name="osb", bufs=8) as out_pool,
            tc.tile_pool(name="pst", bufs=4, space="PSUM") as psum_t,
            tc.tile_pool(name="psm", bufs=4, space="PSUM") as psum_mm,
        ):
            ident = const_pool.tile([P, P], f32, name="ident")
            make_identity(nc, ident)

            _cp = [0]

            def copy(out_ap, in_ap):
                if _cp[0] % 2 == 0:
                    nc.scalar.copy(out_ap, in_ap)
                else:
                    nc.vector.tensor_copy(out=out_ap, in_=in_ap)
                _cp[0] += 1

            # ---- bulk input loads ----
            # X first on both HWDGE rings (4 chunks of 4 m-tiles), then W
            # chunks 0/1; W chunks 2/3 are triggered mid-stream so early DMA
            # bandwidth goes to the data the pipeline start needs.
            XQ = 4  # m-tiles per X chunk
            CO = C_TILES // W_CHUNKS  # 8 c-tiles per chunk
            x_stages = [
                big_pool.tile([P, XQ // 2, 2, L, D], f32, name=f"x_stage{xi}")
                for xi in range(M_TILES // XQ)
            ]
            w_engs = (nc.sync, nc.scalar, nc.sync, nc.scalar)
            w_stages = [
                big_pool.tile([P, CO, L, D], f32, name=f"w_stage{ci}")
                for ci in range(W_CHUNKS)
            ]

            def load_x_chunk(xi, eng):
                # two b-rows per partition: 2KB-contiguous DMA chunks
                eng.dma_start(
                    x_stages[xi],
                    x_dram[xi * XQ * P : (xi + 1) * XQ * P].rearrange(
                        "(mo bi b2) p d -> bi mo b2 p d", bi=P, b2=2
                    ),
                )

            def load_w_chunk(ci):
                w_engs[ci].dma_start(
                    w_stages[ci],
                    w_dram[ci * CO * P : (ci + 1) * CO * P].rearrange(
                        "(co bi) p d -> bi co p d", bi=P
                    ),
                )

            # All loads up front; each ring's FIFO defers the low-priority
            # tail (W2/W3) behind the data the pipeline start needs.
            load_w_chunk(0)
            load_w_chunk(1)
            load_x_chunk(0, nc.sync)
            load_x_chunk(1, nc.scalar)
            load_x_chunk(2, nc.sync)
            load_x_chunk(3, nc.scalar)
            load_w_chunk(2)
            load_w_chunk(3)

            # ---- transposes ----
            # 4 [128,128] PE transposes batched into one PSUM bank, then one
            # [128, 2, 2, 128] strided copy out (cast to f32r).
            def transpose_batch(dst, dst_off, src, src_off, tag, alternate=False):
                ps = psum_t.tile([P, 2, L, P], f32, tag="tps", name=tag)
                for m1 in range(2):
                    for p in range(L):
                        nc.tensor.transpose(
                            ps[:, m1, p, :], src[:, src_off + m1, p, :], ident
                        )
                dst_ap = dst[:, :, dst_off : dst_off + 2 * P].rearrange(
                    "d p (m b) -> d p m b", m=2
                )
                src_ap = ps.rearrange("d m p b -> d p m b")
                if alternate == "scalar":
                    nc.scalar.copy(dst_ap, src_ap)
                else:
                    nc.vector.tensor_copy(out=dst_ap, in_=src_ap)

            # W.T per chunk: wt_chunks[ci][d, p, c'] = W[ci*1024 + c', p, d]
            wt_chunks = [
                big_pool.tile([P, L, C_CHUNK], f32r, name=f"wt{ci}")
                for ci in range(W_CHUNKS)
            ]

            def w_transpose_batch(ci, co2, alternate=False):
                transpose_batch(
                    wt_chunks[ci],
                    co2 * 2 * P,
                    w_stages[ci],
                    co2 * 2,
                    "tps_w",
                    alternate=alternate,
                )

            # chunks 0 and 1 transposed up front (they land first); both
            # copy engines are idle pre-stream, so alternate them here
            for co2 in range(CO // 2):
                w_transpose_batch(0, co2)
            for co2 in range(CO // 2):
                w_transpose_batch(1, co2, alternate="scalar")

            # X.T per chunk: xt_chunks[q][d, p, b'] = X[q*512 + b', p, d]
            xt_chunks = [
                big_pool.tile([P, L, XQ * P], f32r, name=f"xt{xi}")
                for xi in range(M_TILES // XQ)
            ]

            def x_transpose_batch(mo2, alternate=False):
                # batch = (b2, p) for one mo block (256 b's = 2 xt slots)
                xi = mo2 * 2 // XQ
                mo = ((mo2 * 2) % XQ) // 2
                ps = psum_t.tile([P, 2, L, P], f32, tag="tps", name="tps_x")
                for b2 in range(2):
                    for p in range(L):
                        nc.tensor.transpose(
                            ps[:, b2, p, :], x_stages[xi][:, mo, b2, p, :], ident
                        )
                nc.vector.tensor_copy(
                    out=xt_chunks[xi][
                        :, :, mo * 2 * P : (mo * 2 + 2) * P
                    ].rearrange("d p (m b) -> d p m b", m=2),
                    in_=ps.rearrange("d m p b -> d p m b"),
                )

            # only chunk 0 of X (m-tiles 0-3) before the stream; the rest
            # interleave into the early matmul stream below
            x_transpose_batch(0)
            x_transpose_batch(1)

            # ---- main matmul stream: chunk-pair-outer (8KB output rows) ----
            for cpair in range(W_CHUNKS // 2):
                for mt in range(M_TILES):
                    if cpair == 0:
                        # X chunks 1-3 transposes early in the stream (each
                        # well after its stage DMA lands, before first use at
                        # mt 4/8/12); W chunks 2,3 in the back half.
                        if 1 <= mt <= 3:
                            x_transpose_batch(mt * 2)
                            x_transpose_batch(mt * 2 + 1)
                        if mt >= 8:
                            w_transpose_batch(2 + (mt - 8) // 4, (mt - 8) % 4)

                    out_sb = out_pool.tile(
                        [P, 2 * C_CHUNK], f32, tag="osb", name="out_sb"
                    )
                    for sub in range(2):
                        ci = cpair * 2 + sub
                        wt = wt_chunks[ci]
                        pms = [
                            psum_mm.tile([P, N_TILE], f32, tag="pmm", name="pmm")
                            for _ in range(N_GROUP)
                        ]
                        for p in range(L):
                            for j in range(N_GROUP):
                                nc.tensor.matmul(
                                    pms[j],
                                    xt_chunks[mt // XQ][
                                        :, p, (mt % XQ) * P : (mt % XQ + 1) * P
                                    ],
                                    wt[:, p, j * N_TILE : (j + 1) * N_TILE],
                                    start=(p == 0),
                                    stop=(p == L - 1),
                                )
                        for j in range(N_GROUP):
                            copy(
                                out_sb[
                                    :,
                                    sub * C_CHUNK
                                    + j * N_TILE : sub * C_CHUNK
                                    + (j + 1) * N_TILE,
                                ],
                                pms[j],
                            )
                    # xt b-axis is b2-interleaved: out partition bi holds
                    # DRAM row gbase + 2*bi + b2
                    gbase = (mt // 2) * 2 * P
                    b2 = mt % 2
                    nc.sync.dma_start(
                        out_dram[gbase : gbase + 2 * P].rearrange(
                            "(bi b2) c -> b2 bi c", b2=2
                        )[b2, :, cpair * 2 * C_CHUNK : (cpair + 1) * 2 * C_CHUNK],
                        out_sb,
                    )

    nc.compile()
    return nc


def kernel(batchs, label2embed):
    global LAST_RESULT
    from concourse.bass_utils import run_bass_kernel_spmd

    if "nc" not in _CACHE:
        _CACHE["nc"] = _build()
    nc = _CACHE["nc"]

    batchs = np.ascontiguousarray(batchs, dtype=np.float32)
    label2embed = np.ascontiguousarray(label2embed, dtype=np.float32)
    assert batchs.shape == (B, L, D) and label2embed.shape == (C, L, D)

    in_maps = [
        {
            "batchs": batchs[c * B_LOC : (c + 1) * B_LOC],
            "label2embed": label2embed,
        }
        for c in range(N_CORES)
    ]
    res = run_bass_kernel_spmd(
        nc,
        in_maps,
        core_ids=list(range(N_CORES)),
        trace=PROFILE,
        trace_cores=list(range(N_CORES)) if (PROFILE and TRACE_ALL_CORES) else None,
    )
    LAST_RESULT = res
    return np.concatenate([r["out"] for r in res.results], axis=0)


# revision 31
# speedup vs baseline: 1.0009x; 1.0006x over previous
"""Trainium2 Bass kernel for nn_ETypePromptModel: logits = einsum('bpd,cpd->bc').

Equivalent to X @ W.T with X=[B, L*D]=[16384, 256], W=[C, L*D]=[4096, 256].
Data-parallel over B across 8 NeuronCores; label2embed replicated.

Per-core plan (B_LOC=2048), ~112-117us/core measured (DMA-byte bound:
39.5 MB of DRAM traffic per core at the ~420 GB/s sustained fabric rate):
  - All input loads triggered up front: W chunks 0/1 first on the two
    HWDGE rings (sync/scalar), then X in 4 chunks; each ring's FIFO
    defers the W2/W3 tail behind the data the pipeline start needs.
  - PE-transpose X and W into K-major float32r SBUF layout (fp32 has no
    DMA-transpose path); 4 transposes batched per PSUM bank (4 banks) ->
    one [128, 2, 2, 128] strided copy each on the Vector engine. Only the
    start-critical batches (W0, W1, X m-tiles 0-3) run before the matmul
    stream; the rest interleave into it after their stage DMAs land.
  - 256 float32r matmuls ([128k x 128b] stationary, [128k x 512c] moving,
    1 cycle/row vs 4 for plain fp32), K=256 accumulated over 2 PSUM
    passes; groups of 2 PSUM banks (4 banks), chunk-pair-outer stream.
  - PSUM -> SBUF output copies alternate Scalar/Vector; 32 x 1MB HWDGE
    DMA writes (8KB-contiguous rows) of the [2048, 4096] fp32 output
    slice; first write fires ~25us in, stream sustains ~420-427 GB/s.
"""

import sys

import numpy as np

sys.path.insert(0, "/opt/trn_rl_repo")

B, C, L, D = 16384, 4096, 2, 128
N_CORES = 8
B_LOC = B // N_CORES  # 2048
P = 128
N_TILE = 512  # moving free dim per matmul
M_TILES = B_LOC // P  # 16
C_TILES = C // P  # 32
W_CHUNKS = 4
C_CHUNK = C // W_CHUNKS  # 1024 classes per chunk
N_GROUP = 2  # PSUM banks per matmul accumulation group

_CACHE = {}
PROFILE = False
TRACE_ALL_CORES = False
LAST_RESULT = None


def _build():
    import concourse.mybir as mybir
    import concourse.tile as tile
    from concourse import bacc
    from concourse.masks import make_identity

    f32 = mybir.dt.float32
    f32r = mybir.dt.float32r

    nc = bacc.Bacc(
        "TRN2",
        target_bir_lowering=False,
        debug=False,
        enable_asserts=False,
        num_devices=N_CORES,
    )

    x_dram = nc.dram_tensor("batchs", [B_LOC, L, D], f32, kind="ExternalInput").ap()
    w_dram = nc.dram_tensor("label2embed", [C, L, D], f32, kind="ExternalInput").ap()
    out_dram = nc.dram_tensor("out", [B_LOC, C], f32, kind="ExternalOutput").ap()

    with tile.TileContext(nc) as tc:
        with (
            tc.tile_pool(name="const", bufs=1) as const_pool,
            tc.tile_pool(name="big", bufs=1) as big_pool,
            tc.tile_pool(name="osb", bufs=8) as out_pool,
            tc.tile_pool(name="pst", bufs=4, space="PSUM") as psum_t,
            tc.tile_pool(name="psm", bufs=4, space="PSUM") as psum_mm,
        ):
            ident = const_pool.tile([P, P], f32, name="ident")
            make_identity(nc, ident)

            _cp = [0]

            def copy(out_ap, in_ap):
                if _cp[0] % 2 == 0:
                    nc.scalar.copy(out_ap, in_ap)
                else:
                    nc.vector.tensor_copy(out=out_ap, in_=in_ap)
                _cp[0] += 1

            # ---- bulk input loads ----
            # X first on both HWDGE rings (4 chunks of 4 m-tiles), then W
            # chunks 0/1; W chunks 2/3 are triggered mid-stream so early DMA
            # bandwidth goes to the data the pipeline start needs.
            XQ = 4  # m-tiles per X chunk
            CO = C_TILES // W_CHUNKS  # 8 c-tiles per chunk
            x_stages = [
                big_pool.tile([P, XQ // 2, 2, L, D], f32, name=f"x_stage{xi}")
                for xi in range(M_TILES // XQ)
            ]
            w_engs = (nc.sync, nc.scalar, nc.sync, nc.scalar)
            w_stages = [
                big_pool.tile([P, CO, L, D], f32, name=f"w_stage{ci}")
                for ci in range(W_CHUNKS)
            ]

            def load_x_chunk(xi, eng):
                # two b-rows per partition: 2KB-contiguous DMA chunks
                eng.dma_start(
                    x_stages[xi],
                    x_dram[xi * XQ * P : (xi + 1) * XQ * P].rearrange(
                        "(mo bi b2) p d -> bi mo b2 p d", bi=P, b2=2
                    ),
                )

            def load_w_chunk(ci):
                w_engs[ci].dma_start(
                    w_stages[ci],
                    w_dram[ci * CO * P : (ci + 1) * CO * P].rearrange(
                        "(co bi) p d -> bi co p d", bi=P
                    ),
                )

            # All loads up front; each ring's FIFO defers the low-priority
            # tail (W2/W3) behind the data the pipeline start needs.
            load_w_chunk(0)
            load_w_chunk(1)
            load_x_chunk(0, nc.sync)
            load_x_chunk(1, nc.scalar)
            load_x_chunk(2, nc.sync)
            load_x_chunk(3, nc.scalar)
            load_w_chunk(2)
            load_w_chunk(3)

            # ---- transposes ----
            # 4 [128,128] PE transposes batched into one PSUM bank, then one
            # [128, 2, 2, 128] strided copy out (cast to f32r).
            def transpose_batch(dst, dst_off, src, src_off, tag, alternate=False):
                ps = psum_t.tile([P, 2, L, P], f32, tag="tps", name=tag)
                for m1 in range(2):
                    for p in range(L):
                        nc.tensor.transpose(
                            ps[:, m1, p, :], src[:, src_off + m1, p, :], ident
                        )
                dst_ap = dst[:, :, dst_off : dst_off + 2 * P].rearrange(
                    "d p (m b) -> d p m b", m=2
                )
                src_ap = ps.rearrange("d m p b -> d p m b")
                if alternate == "scalar":
                    nc.scalar.copy(dst_ap, src_ap)
                else:
                    nc.vector.tensor_copy(out=dst_ap, in_=src_ap)

            # W.T per chunk: wt_chunks[ci][d, p, c'] = W[ci*1024 + c', p, d]
            wt_chunks = [
                big_pool.tile([P, L, C_CHUNK], f32r, name=f"wt{ci}")
                for ci in range(W_CHUNKS)
            ]

            def w_transpose_batch(ci, co2, alternate=False):
                transpose_batch(
                    wt_chunks[ci],
                    co2 * 2 * P,
                    w_stages[ci],
                    co2 * 2,
                    "tps_w",
                    alternate=alternate,
                )

            # chunks 0 and 1 transposed up front (they land first); both
            # copy engines are idle pre-stream, so alternate them here
            for co2 in range(CO // 2):
                w_transpose_batch(0, co2)
            for co2 in range(CO // 2):
                w_transpose_batch(1, co2, alternate="scalar")

            # X.T per chunk: xt_chunks[q][d, p, b'] = X[q*512 + b', p, d]
            xt_chunks = [
                big_pool.tile([P, L, XQ * P], f32r, name=f"xt{xi}")
                for xi in range(M_TILES // XQ)
            ]

            def x_transpose_batch(mo2, alternate=False):
                # batch = (b2, p) for one mo block (256 b's = 2 xt slots)
                xi = mo2 * 2 // XQ
                mo = ((mo2 * 2) % XQ) // 2
                ps = psum_t.tile([P, 2, L, P], f32, tag="tps", name="tps_x")
                for b2 in range(2):
                    for p in range(L):
                        nc.tensor.transpose(
                            ps[:, b2, p, :], x_stages[xi][:, mo, b2, p, :], ident
                        )
                nc.vector.tensor_copy(
                    out=xt_chunks[xi][
                        :, :, mo * 2 * P : (mo * 2 + 2) * P
                    ].rearrange("d p (m b) -> d p m b", m=2),
                    in_=ps.rearrange("d m p b -> d p m b"),
                )

            # only chunk 0 of X (m-tiles 0-3) before the stream; the rest
            # interleave into the early matmul stream below
            x_transpose_batch(0)
            x_transpose_batch(1)

            # ---- main matmul stream: chunk-pair-outer (8KB output rows) ----
            for cpair in range(W_CHUNKS // 2):
                for mt in range(M_TILES):
                    if cpair == 0:
                        # X chunks 1-3 transposes early in the stream (each
                        # well after its stage DMA lands, before first use at
                        # mt 4/8/12); W chunks 2,3 in the back half.
                        if 1 <= mt <= 3:
                            x_transpose_batch(mt * 2)
                            x_transpose_batch(mt * 2 + 1)
                        if mt >= 8:
                            w_transpose_batch(2 + (mt - 8) // 4, (mt - 8) % 4)

                    out_sb = out_pool.tile(
                        [P, 2 * C_CHUNK], f32, tag="osb", name="out_sb"
                    )
                    for sub in range(2):
                        ci = cpair * 2 + sub
                        wt = wt_chunks[ci]
                        pms = [
                            psum_mm.tile([P, N_TILE], f32, tag="pmm", name="pmm")
                            for _ in range(N_GROUP)
                        ]
                        for p in range(L):
                            for j in range(N_GROUP):
                                nc.tensor.matmul(
                                    pms[j],
                                    xt_chunks[mt // XQ][
                                        :, p, (mt % XQ) * P : (mt % XQ + 1) * P
                                    ],
                                    wt[:, p, j * N_TILE : (j + 1) * N_TILE],
                                    start=(p == 0),
                                    stop=(p == L - 1),
                                )
                        for j in range(N_GROUP):
                            copy(
                                out_sb[
                                    :,
                                    sub * C_CHUNK
                                    + j * N_TILE : sub * C_CHUNK
                                    + (j + 1) * N_TILE,
                                ],
                                pms[j],
                            )
                    # xt b-axis is b2-interleaved: out partition bi holds
                    # DRAM row gbase + 2*bi + b2
                    gbase = (mt // 2) * 2 * P
                    b2 = mt % 2
                    nc.sync.dma_start(
                        out_dram[gbase : gbase + 2 * P].rearrange(
                            "(bi b2) c -> b2 bi c", b2=2
                        )[b2, :, cpair * 2 * C_CHUNK : (cpair + 1) * 2 * C_CHUNK],
                        out_sb,
                    )

    nc.compile()
    return nc


def kernel(batchs, label2embed):
    global LAST_RESULT
    from concourse.bass_utils import run_bass_kernel_spmd

    if "nc" not in _CACHE:
        _CACHE["nc"] = _build()
    nc = _CACHE["nc"]

    batchs = np.ascontiguousarray(batchs, dtype=np.float32)
    label2embed = np.ascontiguousarray(label2embed, dtype=np.float32)
    assert batchs.shape == (B, L, D) and label2embed.shape == (C, L, D)

    in_maps = [
        {
            "batchs": batchs[c * B_LOC : (c + 1) * B_LOC],
            "label2embed": label2embed,
        }
        for c in range(N_CORES)
    ]
    res = run_bass_kernel_spmd(
        nc,
        in_maps,
        core_ids=list(range(N_CORES)),
        trace=PROFILE,
        trace_cores=list(range(N_CORES)) if (PROFILE and TRACE_ALL_CORES) else None,
    )
    LAST_RESULT = res
    return np.concatenate([r["out"] for r in res.results], axis=0)
